# revision 1
# baseline (speedup 1.0000x reference)
"""GNN message-passing kernel for Trainium2 (8 NeuronCores).

out[v] = tanh( sum_w W[w] @ sum_{edges e: v_e=v, widx_e=w} x[u_e] )

Strategy (dest-sharded, no collectives):
  - Nodes (destinations) sharded across 8 cores: core c owns v in
    [c*12500, (c+1)*12500).  Edges bucketed host-side by
    (dest_core, widx, u_window, occurrence_rank) -- sharding/layout step.
  - Per core: Phase Z zeroes per-w segment tables S_w [SRW, D].
    Phase G/S: per (w, u_window): one dma_gather of x[u] rows (int16
    window-local indices) into SBUF staging, then one dma_scatter_add
    per occurrence-rank batch into S_w rows v_local.  Rank batching
    guarantees indices are unique within each scatter instruction
    (HW CCE-add races on duplicates); batches serialize via Tile's
    per-tensor tracking.  Pad slots scatter into a trash row.
    Phase D: per 128-node tile: load S_w tiles, PE-transpose,
    matmul-accumulate against W^T over w, tanh, store.
  - Output: concat of per-core 12500-row slices (host-side unshard).
"""
import os
import numpy as np

import concourse.bass as bass
import concourse.bacc as bacc
import concourse.mybir as mybir
import concourse.tile as tile
from concourse.bass_utils import run_bass_kernel_spmd
from concourse.masks import make_identity

# problem shape (hardcoded per contract)
N, D, E, NW = 100000, 128, 2000000, 8
C = 8                  # cores
NPC = N // C           # 12500 nodes per core
WIN = 32768            # u gather window (int16-addressable rows)
NWIN = 4               # ceil(N / WIN)
SRW = 12800            # S rows per w (98*128=12544 read by dense + trash)
TRASH = 12600          # scatter pad target row (never read by dense phase)
NTILE = 98             # dense-phase node tiles of 128 (12544 rows)

# occurrence-rank batch capacities (multiples of 128), sized from the actual
# seed-0 edge distribution with margin.  win 0-2 see ~10.3k edges per
# (core,w,win); win 3 covers only 1696 source rows.
RANK_CAPS_W012 = [7424, 2816, 896, 256, 128, 128, 128, 128, 128]  # sum 12032
RANK_CAPS_W3 = [640, 128, 128, 128]                               # sum 1024
SLICE_CAPS = [RANK_CAPS_W012] * 3 + [RANK_CAPS_W3]
SLICE_TOT = [sum(cs) for cs in SLICE_CAPS]      # slots per (w, win)
PER_W = sum(SLICE_TOT)                          # 3*12032 + 1024 = 37120
TOT = NW * PER_W                                # slots per core

LAST_RESULTS = None    # BassKernelResults of the most recent run (for profiling)


def _wrap16(flat):
    """[n] -> [128, n/16] idx layout: position i at [i%16, i//16], replicated 8x."""
    base = flat.reshape(-1, 16).T  # [16, n/16]
    return np.tile(base, (8, 1))


def _build_nc():
    nc = bacc.Bacc("TRN2", target_bir_lowering=False, debug=False, num_devices=C,
                   num_swdge_queues=4)
    x_d = nc.dram_tensor("x", [N, D], mybir.dt.float32, kind="ExternalInput")
    wt_d = nc.dram_tensor("wt", [NW, D, D], mybir.dt.float32, kind="ExternalInput")
    gidx_d = nc.dram_tensor("gidx", [128, TOT // 16], mybir.dt.int16, kind="ExternalInput")
    sidx_d = nc.dram_tensor("sidx", [128, TOT // 16], mybir.dt.int16, kind="ExternalInput")
    out_d = nc.dram_tensor("out", [NTILE * 128, D], mybir.dt.float32, kind="ExternalOutput")
    s_w = [nc.dram_tensor(f"S{w}", [SRW, D], mybir.dt.float32) for w in range(NW)]

    with tile.TileContext(nc) as tc:
        # ---- Phase Z: zero all S_w ----
        with tc.tile_pool(name="zpool", bufs=1) as zpool:
            zt = zpool.tile([128, 1600], mybir.dt.float32)
            nc.vector.memset(zt[:], 0.0)
            for w in range(NW):
                sv = s_w[w][:].rearrange("r d -> (r d)").rearrange(
                    "(b p q) -> b p q", p=128, q=1600)
                for b in range(8):
                    nc.sync.dma_start(out=sv[b], in_=zt[:])

            # ---- Phase G/S ----
            with (
                tc.tile_pool(name="idxp", bufs=3) as idxp,
                tc.tile_pool(name="stg", bufs=3) as stg,
            ):
                off = 0
                for w in range(NW):
                    for win in range(NWIN):
                        caps = SLICE_CAPS[win]
                        cap = SLICE_TOT[win]
                        lo, hi = win * WIN, min((win + 1) * WIN, N)
                        gi = idxp.tile([128, cap // 16], mybir.dt.int16, tag="gi")
                        nc.sync.dma_start(
                            out=gi[:], in_=gidx_d[:, off // 16:(off + cap) // 16])
                        si = idxp.tile([128, cap // 16], mybir.dt.int16, tag="si")
                        nc.sync.dma_start(
                            out=si[:], in_=sidx_d[:, off // 16:(off + cap) // 16])
                        st = stg.tile([128, SLICE_TOT[0] // 128, D],
                                      mybir.dt.float32)
                        stv = st[:, :cap // 128, :]
                        nc.gpsimd.dma_gather(
                            stv, x_d[lo:hi], gi[:], cap, cap, D,
                            single_packet=False, queue_num=2 + w % 2)
                        co = 0  # column offset within this slice
                        for bcap in caps:
                            nc.gpsimd.dma_scatter_add(
                                s_w[w][:],
                                st[:, co // 128:(co + bcap) // 128, :],
                                si[:, co // 16:(co + bcap) // 16],
                                bcap, bcap, D,
                                single_packet=False, queue_num=w % 2)
                            co += bcap
                        off += cap

        # ---- Phase D: out = tanh(sum_w S_w @ W_w^T) ----
        with (
            tc.tile_pool(name="const", bufs=1) as constp,
            tc.tile_pool(name="dense", bufs=3) as dense,
            tc.tile_pool(name="psum", bufs=4, space="PSUM") as psum,
        ):
            ident = constp.tile([128, 128], mybir.dt.float32)
            make_identity(nc, ident[:])
            wt = constp.tile([128, NW, D], mybir.dt.float32)
            nc.sync.dma_start(out=wt[:], in_=wt_d[:].rearrange("w j i -> j w i"))
            for t in range(NTILE):
                sload = dense.tile([128, NW, D], mybir.dt.float32)
                for w in range(NW):
                    nc.sync.dma_start(
                        out=sload[:, w, :], in_=s_w[w][t * 128:(t + 1) * 128, :])
                outp = psum.tile([128, 128], mybir.dt.float32, tag="acc")
                for w in range(NW):
                    tp = psum.tile([128, 128], mybir.dt.float32, tag="tp")
                    nc.tensor.transpose(out=tp[:], in_=sload[:, w, :], identity=ident[:])
                    ts = dense.tile([128, 128], mybir.dt.float32, tag="ts")
                    nc.vector.tensor_copy(out=ts[:], in_=tp[:])
                    nc.tensor.matmul(
                        out=outp[:], lhsT=ts[:], rhs=wt[:, w, :],
                        start=(w == 0), stop=(w == NW - 1))
                ot = dense.tile([128, 128], mybir.dt.float32, tag="ot")
                nc.scalar.activation(ot[:], outp[:], mybir.ActivationFunctionType.Tanh)
                nc.sync.dma_start(out=out_d[t * 128:(t + 1) * 128, :], in_=ot[:])

    nc.compile()
    return nc


def _prep_cores(u, v, widx):
    """Bucket edges by (core, w, u_window, occurrence-rank); build idx arrays."""
    c = v // NPC
    uwin = u // WIN
    vloc = v - c * NPC
    bucket = (c * NW + widx) * NWIN + uwin
    # occurrence rank of (bucket, vloc)
    pair = bucket * NPC + vloc
    o1 = np.argsort(pair, kind="stable")
    ps = pair[o1]
    isnew = np.ones(len(ps), bool)
    isnew[1:] = ps[1:] != ps[:-1]
    run_id = np.cumsum(isnew) - 1
    run_starts = np.flatnonzero(isnew)
    rank = np.arange(len(ps)) - run_starts[run_id]
    # order edges by (bucket, rank)
    k2 = bucket[o1] * 16 + rank
    o2 = np.argsort(k2, kind="stable")
    eid = o1[o2]               # edge ids in final order
    k2s = k2[o2]
    # counts per (bucket, rank)
    nb = C * NW * NWIN
    cnt = np.bincount(k2s, minlength=nb * 16).reshape(nb, 16)
    gidx_all, sidx_all = [], []
    gsrc = (u - uwin * WIN).astype(np.int16)
    ssrc = vloc.astype(np.int16)
    pos = np.concatenate([[0], np.cumsum(cnt.reshape(-1))[:-1]]).reshape(nb, 16)
    for cc in range(C):
        g_flat = np.zeros(TOT, np.int16)
        s_flat = np.full(TOT, TRASH, np.int16)
        off = 0
        for w in range(NW):
            for win in range(NWIN):
                caps = SLICE_CAPS[win]
                b = (cc * NW + w) * NWIN + win
                nrank = cnt[b]
                assert nrank[len(caps):].sum() == 0, (
                    f"bucket {b} has ranks beyond {len(caps)}: {nrank}")
                for k, bcap in enumerate(caps):
                    n = int(nrank[k])
                    assert n <= bcap, f"bucket {b} rank {k}: {n} > {bcap}"
                    sel = eid[pos[b, k]:pos[b, k] + n]
                    g_flat[off:off + n] = gsrc[sel]
                    s_flat[off:off + n] = ssrc[sel]
                    off += bcap
        assert off == TOT
        gidx_all.append(_wrap16(g_flat))
        sidx_all.append(_wrap16(s_flat))
    return gidx_all, sidx_all


def kernel(x, W, u, v, widx):
    global LAST_RESULTS
    x = np.ascontiguousarray(np.asarray(x, dtype=np.float32))
    W = np.asarray(W, dtype=np.float32)
    u = np.asarray(u).astype(np.int64)
    v = np.asarray(v).astype(np.int64)
    widx = np.asarray(widx).astype(np.int64)

    gidx_all, sidx_all = _prep_cores(u, v, widx)
    wt_np = np.ascontiguousarray(np.transpose(W, (0, 2, 1)))  # W_T[w] = W[w].T

    nc = _build_nc()
    in_maps = [
        {"x": x, "wt": wt_np, "gidx": gidx_all[cc], "sidx": sidx_all[cc]}
        for cc in range(C)
    ]

    trace = bool(os.environ.get("KERNEL_TRACE"))
    LAST_RESULTS = run_bass_kernel_spmd(
        nc, in_maps, core_ids=list(range(C)),
        trace=trace, trace_cores=[0] if trace else None,
    )
    out = np.concatenate(
        [LAST_RESULTS.results[cc]["out"][:NPC] for cc in range(C)], axis=0)
    return out.astype(np.float32)



# revision 4
# speedup vs baseline: 2.2143x; 2.2143x over previous
"""GNN message-passing kernel for Trainium2 (8 NeuronCores).

out[v] = tanh( sum_w W[w] @ sum_{edges e: v_e=v, widx_e=w} x[u_e] )

Strategy (dest-sharded, PSUM-accumulated segment sums, no scatter):
  - Nodes (destinations) sharded across 8 cores: core c owns v in
    [c*12500, (c+1)*12500) = 98 dest tiles of 128 rows.
  - Edges bucketed host-side into cells (dest_tile t, u_window win,
    weight w) with a fixed capacity of 128 slots per cell (observed max
    117 with balanced 25000-row u-windows).  Pad slots gather x[win*W]
    (index 0) and are zeroed by the routing matmul.
  - Per core: per gather-block (6 dest tiles x 4 windows): one
    dma_gather of 6144 x rows (fp32, window-local int16 indices) into
    SBUF; DVE converts to fp16; DVE builds one-hot routing matrices
    P[e, r] = (vloc[e] == r) via is_equal against an iota constant.
  - Segment sums accumulate directly in PSUM: per cell one matmul
    S^T[j, r] += sum_e X[e, j] * P[e, r]  (lhsT=X chunk, rhs=P chunk,
    contraction over the 128 slot-partitions).  The 8 w-accumulators of
    a dest tile live in 2 PSUM banks; psum groups span the 4 windows
    (start on win 0, stop on win 3).  3 tiles in flight = 6 banks.
  - Dense stage per tile: copy the 2 PSUM regions to SBUF fp16, 8
    matmuls out^T[i, r] += wt[j, w, i]^T @ S^T[j, r] accumulating over
    w, tanh on Scalar, write out^T (fp16) to DRAM.
  - Output: per-core [128, 12544] out^T; host transposes, trims to
    12500 rows, concats cores, casts fp32.
"""
import os
import numpy as np

import concourse.bass as bass
import concourse.bacc as bacc
import concourse.mybir as mybir
import concourse.tile as tile
from concourse.bass_utils import run_bass_kernel_spmd

# problem shape (hardcoded per contract)
N, D, E, NW = 100000, 128, 2000000, 8
C = 8                  # cores
NPC = N // C           # 12500 dest nodes per core
WIN = 25000            # balanced u-window size (int16-addressable)
NWIN = 4
NT = 98                # dest tiles of 128 (12544 >= 12500)
CAP = 128              # slots per (t, win, w) cell
GT = 6                 # dest tiles per gather block
TG = 3                 # dest tiles per psum batch (6 of 8 psum banks)

# gather blocks: (first_tile, n_tiles)
GBS = [(i, min(GT, NT - i)) for i in range(0, NT, GT)]
# slot base of each (gb, win) block
_BASES = {}
_off = 0
for _gi, (_t0, _nt) in enumerate(GBS):
    for _w in range(NWIN):
        _BASES[(_gi, _w)] = _off
        _off += _nt * 8 * CAP
TOT = _off             # 401408 slots per core
PAD_VKEY = 300.0       # vkey for pad slots: matches no dest row 0..127

LAST_RESULTS = None    # BassKernelResults of the most recent run (for profiling)


def _wrap16(flat):
    """[n] -> [128, n/16] idx layout: position i at [i%16, i//16], replicated 8x."""
    base = flat.reshape(-1, 16).T  # [16, n/16]
    return np.tile(base, (8, 1))


def _build_nc():
    f32, f16, i16 = mybir.dt.float32, mybir.dt.float16, mybir.dt.int16
    nc = bacc.Bacc("TRN2", target_bir_lowering=False, debug=False, num_devices=C,
                   num_swdge_queues=4)
    x_d = nc.dram_tensor("x", [N, D], f32, kind="ExternalInput")
    wt_d = nc.dram_tensor("wt", [NW, D, D], f32, kind="ExternalInput")
    gidx_d = nc.dram_tensor("gidx", [128, TOT // 16], i16, kind="ExternalInput")
    vk_d = nc.dram_tensor("vk", [128, TOT // 128], f16, kind="ExternalInput")
    out_d = nc.dram_tensor("out", [D, NT * 128], f16, kind="ExternalOutput")

    GBC = GT * 8       # max columns per (gb, win) block

    with tile.TileContext(nc) as tc:
        with tc.tile_pool(name="const", bufs=1) as const:
            # iota constant: iota16[p, c, r] = r  (fp16, exact for 0..127)
            iota16 = const.tile([128, GBC, 128], f16)
            wt16 = const.tile([128, NW, D], f16)
            with tc.tile_pool(name="init", bufs=1) as init:
                ii = init.tile([128, GBC, 128], i16)
                nc.gpsimd.iota(ii[:], pattern=[[0, GBC], [1, 128]], base=0,
                               channel_multiplier=0)
                nc.vector.tensor_copy(out=iota16[:], in_=ii[:])
                wtf = init.tile([128, NW, D], f32)
                nc.sync.dma_start(out=wtf[:], in_=wt_d[:].rearrange("w j i -> j w i"))
                nc.vector.tensor_copy(out=wt16[:], in_=wtf[:])

            with (
                tc.tile_pool(name="gip", bufs=3) as gip,
                tc.tile_pool(name="vlp", bufs=3) as vlp,
                tc.tile_pool(name="stgf", bufs=2) as stgfp,
                tc.tile_pool(name="stg16", bufs=4) as stg16p,
                tc.tile_pool(name="pp", bufs=4) as pp,
                tc.tile_pool(name="s16p", bufs=3) as s16p,
                tc.tile_pool(name="outp", bufs=3) as outp,
                tc.tile_pool(name="sreg", bufs=1, space="PSUM") as sregp,
                tc.tile_pool(name="wout", bufs=2, space="PSUM") as woutp,
            ):
                for gbi, (t0, nt) in enumerate(GBS):
                    bc = nt * 8            # columns in this block
                    nidx = bc * CAP        # slots per (gb, win)
                    stg_w, p_w = [], []
                    for win in range(NWIN):
                        base = _BASES[(gbi, win)]
                        gi = gip.tile([128, GBC * 8], i16, tag="gi")
                        nc.sync.dma_start(
                            out=gi[:, :nidx // 16],
                            in_=gidx_d[:, base // 16:(base + nidx) // 16])
                        vl = vlp.tile([128, GBC], f16, tag="vl")
                        nc.sync.dma_start(
                            out=vl[:, :bc],
                            in_=vk_d[:, base // 128:(base + nidx) // 128])
                        stgf = stgfp.tile([128, GBC, 128], f32, tag="stgf")
                        nc.gpsimd.dma_gather(
                            stgf[:, :bc, :], x_d[win * WIN:(win + 1) * WIN],
                            gi[:, :nidx // 16], nidx, nidx, D,
                            single_packet=False, queue_num=(gbi * NWIN + win) % 4)
                        stg = stg16p.tile([128, GBC, 128], f16, tag="stg")
                        nc.vector.tensor_copy(out=stg[:, :bc, :], in_=stgf[:, :bc, :])
                        pt = pp.tile([128, GBC, 128], f16, tag="pt")
                        nc.vector.tensor_tensor(
                            out=pt[:, :bc, :],
                            in0=vl[:, :bc].to_broadcast([128, bc, 128]),
                            in1=iota16[:, :bc, :],
                            op=mybir.AluOpType.is_equal)
                        stg_w.append(stg)
                        p_w.append(pt)

                    for sb in range((nt + TG - 1) // TG):
                        tls = range(sb * TG, min(sb * TG + TG, nt))
                        # psum regions: 2 banks per tile in flight
                        sregs = {
                            (tl, h): sregp.tile([128, 4, 128], f32,
                                                tag=f"sr{tl - sb * TG}{h}",
                                                name=f"sr{tl - sb * TG}{h}")
                            for tl in tls for h in range(2)
                        }
                        for win in range(NWIN):
                            for tl in tls:
                                for w in range(NW):
                                    col = tl * 8 + w
                                    reg = sregs[(tl, w // 4)]
                                    nc.tensor.matmul(
                                        out=reg[:, w % 4, :],
                                        lhsT=stg_w[win][:, col, :],
                                        rhs=p_w[win][:, col, :],
                                        start=(win == 0 and w % 4 == 0),
                                        stop=(win == NWIN - 1 and w % 4 == 3))
                        for tl in tls:
                            gt = t0 + tl
                            s16 = s16p.tile([128, NW, D], f16, tag="s16")
                            nc.vector.tensor_copy(out=s16[:, 0:4, :],
                                                  in_=sregs[(tl, 0)][:])
                            nc.vector.tensor_copy(out=s16[:, 4:8, :],
                                                  in_=sregs[(tl, 1)][:])
                            wo = woutp.tile([128, 4, 128], f32, tag="wo")
                            for w in range(NW):
                                nc.tensor.matmul(
                                    out=wo[:, 0, :], lhsT=wt16[:, w, :],
                                    rhs=s16[:, w, :],
                                    start=(w == 0), stop=(w == NW - 1))
                            ot = outp.tile([128, 128], f16, tag="ot")
                            nc.scalar.activation(ot[:], wo[:, 0, :],
                                                 mybir.ActivationFunctionType.Tanh)
                            nc.sync.dma_start(
                                out=out_d[:, gt * 128:(gt + 1) * 128], in_=ot[:])

    nc.compile()
    return nc


def _prep_cores(u, v, widx):
    """Bucket edges into (core, dest_tile, u_window, w) cells of 128 slots."""
    c = v // NPC
    vloc = v - c * NPC
    t = vloc // 128
    r = vloc % 128
    win = u // WIN
    gl = (u - win * WIN).astype(np.int16)
    gbi = t // GT
    tl = t - gbi * GT

    # rank within cell (c, t, win, w)
    cell = ((c * NT + t) * NWIN + win) * NW + widx
    o = np.argsort(cell, kind="stable")
    cs = cell[o]
    isnew = np.ones(len(cs), bool)
    isnew[1:] = cs[1:] != cs[:-1]
    run_id = np.cumsum(isnew) - 1
    run_starts = np.flatnonzero(isnew)
    rank = np.arange(len(cs)) - run_starts[run_id]
    rank_e = np.empty(len(cs), np.int64)
    rank_e[o] = rank

    maxrank = np.zeros(C, np.int64)
    np.maximum.at(maxrank, c, rank_e)
    assert rank_e.max() < CAP, f"cell overflow: max rank {rank_e.max()}"

    base_arr = np.zeros((len(GBS), NWIN), np.int64)
    for k, b in _BASES.items():
        base_arr[k] = b
    slot = base_arr[gbi, win] + (tl * 8 + widx) * CAP + rank_e

    gidx_all, vk_all = [], []
    for cc in range(C):
        m = c == cc
        g_flat = np.zeros(TOT, np.int16)
        v_flat = np.full(TOT, PAD_VKEY, np.float16)
        g_flat[slot[m]] = gl[m]
        v_flat[slot[m]] = r[m].astype(np.float16)
        gidx_all.append(_wrap16(g_flat))
        vk_all.append(np.ascontiguousarray(v_flat.reshape(-1, 128).T))
    return gidx_all, vk_all


def kernel(x, W, u, v, widx):
    global LAST_RESULTS
    x = np.ascontiguousarray(np.asarray(x, dtype=np.float32))
    W = np.asarray(W, dtype=np.float32)
    u = np.asarray(u).astype(np.int64)
    v = np.asarray(v).astype(np.int64)
    widx = np.asarray(widx).astype(np.int64)

    gidx_all, vk_all = _prep_cores(u, v, widx)
    # wt[w, j, i] = W[w, i, j]
    wt_np = np.ascontiguousarray(np.transpose(W, (0, 2, 1)))

    nc = _build_nc()
    in_maps = [
        {"x": x, "wt": wt_np, "gidx": gidx_all[cc], "vk": vk_all[cc]}
        for cc in range(C)
    ]

    trace = bool(os.environ.get("KERNEL_TRACE"))
    LAST_RESULTS = run_bass_kernel_spmd(
        nc, in_maps, core_ids=list(range(C)),
        trace=trace, trace_cores=[0] if trace else None,
    )
    out = np.concatenate(
        [np.asarray(LAST_RESULTS.results[cc]["out"]).T[:NPC] for cc in range(C)],
        axis=0)
    return out.astype(np.float32)
